# revision 4
# baseline (speedup 1.0000x reference)
"""TRN2 Bass kernel for nn_RF_ANFISModel (ANFIS forward pass).

Model (per batch row b):
  memb[b,v,m] = 1/(1+((x[b,v]-c[v,m])/a[v,m])^2 ^ b[v,m])
  rules[b,r]  = prod_v memb[b, v, mf_indices[r,v]]
  out[b]      = sum_r rules*(coeff[r,0,:].[x;1]) / max(sum_r rules, 1e-12)

Strategy: pure data parallelism over the batch (8 NeuronCores x 512 rows).
The rule product is computed in log space: rules = exp(-(S @ Lpos)) with
Lpos[(v,m),b] = ln(1+dist^b) and S a one-hot selection matrix.

Fast path (mf_indices == full lexicographic product, as produced by
setup_inputs): rules[r] factorizes as A2[hi2]*B4[lo4] with hi2=(i1,i2)
(16 values) and lo4=(i3..i6) (256 values). The coeff contraction
G[j,b] = sum_r coeff_aug[r,j]*rules[r,b] is restructured as
  T[(hi2,j),b]   = sum_lo4 C4[lo4,(hi2,j)] * B4[lo4,b]      (PE, K=256)
  ABig[(hi2,j),b]= A2[hi2,b]  (broadcast via one-hot matmul)
  U = T*ABig ; G[j,b] = sum_hi2 U[(hi2,j),b] ; V = G*xp8
and the host finishes with out = sum_j<7 V[j] / max(V[7], 1e-12).

Generic fallback (arbitrary mf_indices / non-2.0 exponents) does the
full one-hot log-space matmul over 32 rule tiles.

Value-domain matmuls run in float32r (~12-bit mantissa rounding of the
operands, fp32 accumulation): measured end-to-end max error ~3e-4 of the
output scale. Log-domain one-hot matmuls use a bf16 hi/lo split in the
generic path.
"""
import numpy as np

import concourse.mybir as mybir
import concourse.tile as tile
from concourse import bacc
import bass_rust as _bass_rust
from concourse.hw_specs import get_activation_tables
from concourse.bass_utils import run_bass_kernel_spmd

F32 = mybir.dt.float32
F32R = mybir.dt.float32r
BF16 = mybir.dt.bfloat16
AF = mybir.ActivationFunctionType
ALU = mybir.AluOpType

NIN, NMF, NQ, R = 6, 4, 24, 4096
NCORES, B, BL = 8, 4096, 512


class _BaccOneActSet(bacc.Bacc):
    """Bacc whose act-table pass only sees `natural_log_exp_and_others`
    (true set index preserved), so Ln/Exp/Copy/Square share one table
    load instead of thrashing between `natural_log` and
    `exp_and_others`."""

    _ONE_SET = "natural_log_exp_and_others"

    def insert_act_table_loads(self):
        has_activation = any(
            isinstance(i, mybir.InstActivation)
            for b in self.main_func.blocks
            for i in b.instructions
        )
        if not has_activation:
            return
        tables = [
            (name, funcs if name == self._ONE_SET else set())
            for name, funcs in get_activation_tables(self.m.arch).items()
        ]
        _bass_rust.insert_act_table_loads(self, tables)


# ---------------------------------------------------------------------------
# host-side prep
# ---------------------------------------------------------------------------

def _pad48(arr24):
    """[24, N] -> [48, N]: q-rows for v0,v1 at partitions 0:8; v2..v5 at
    32:48 (engine partition reads must be 32-aligned)."""
    out = np.tile(arr24[0:1], (48, 1)).astype(np.float32)
    out[0:8] = arr24[0:8]
    out[32:48] = arr24[8:24]
    return np.ascontiguousarray(out)


def _is_lex_product(mf_indices):
    r = np.arange(R)
    want = np.stack(
        [(r >> (2 * (NIN - 1 - v))) & 3 for v in range(NIN)], axis=1)
    return mf_indices.shape == (R, NIN) and np.array_equal(
        np.asarray(mf_indices).astype(np.int64), want)


def _prep_common(x, a, b, c, coeff):
    xT = np.ascontiguousarray(np.asarray(x, np.float32).T)     # [6, B]
    xT24 = np.repeat(xT, NMF, axis=0)                          # [24, B]
    cvec = np.asarray(c, np.float32).reshape(NQ, 1)
    ainv = (1.0 / np.asarray(a, np.float32)).reshape(NQ, 1)
    bexp = np.asarray(b, np.float32).reshape(NQ, 1)
    xp8 = np.ones((8, xT.shape[1]), np.float32)
    xp8[:NIN] = xT
    coeff_aug = np.ones((R, 8), np.float32)
    coeff_aug[:, :7] = np.asarray(coeff, np.float32).reshape(R, 7)
    return dict(xT24=xT24, cvec=cvec, ainv=ainv, bexp=bexp, xp8=xp8,
                coeff_aug=coeff_aug,
                x48=_pad48(xT24), cvec48=_pad48(cvec), ainv48=_pad48(ainv),
                bexp48=_pad48(bexp))


def _prep_fast16(prep):
    coeff_aug = prep["coeff_aug"]
    # wf [128, 394] f32r: cols 0:256 C4 (kk-major), 256:384 ecat2,
    # 384:392 red, 392:394 unused
    wf = np.zeros((128, 394), np.float32)
    c4 = coeff_aug.reshape(16, 2, 128, 8)   # [hi2, kk, lo4-within-chunk, j]
    for kk in range(2):
        for hi2 in range(16):
            for j in range(8):
                wf[:, kk * 128 + hi2 * 8 + j] = c4[hi2, kk, :, j]
    for hi2 in range(16):
        wf[hi2, 256 + hi2 * 8:256 + hi2 * 8 + 8] = 1.0
    wf[:, 384:392] = np.tile(np.eye(8, dtype=np.float32), (16, 1))

    # wb [48, 272]: rows 0:8 cols 0:16 s3a2 one-hot; rows 32:48 cols
    # 16:272 s3b4 one-hot
    wb = np.zeros((48, 272), np.float32)
    hi2 = np.arange(16)
    for v in range(2):
        dig = (hi2 >> (2 * (1 - v))) & 3
        for m in range(NMF):
            wb[v * 4 + m, np.where(dig == m)[0]] = 1.0
    lo4 = np.arange(256)
    for v in range(4):
        dig = (lo4 >> (2 * (3 - v))) & 3
        for m in range(NMF):
            wb[32 + v * 4 + m, 16 + np.where(dig == m)[0]] = 1.0
    return dict(wf=wf, wb=wb)


def _make_pcore(prep, sl):
    p = np.zeros((48, 516), np.float32)
    p[:, 0:512] = prep["x48"][:, sl]
    p[:, 512] = prep["cvec48"][:, 0]
    p[:, 513] = prep["ainv48"][:, 0]
    p[:, 514] = prep["bexp48"][:, 0]
    p[:, 515] = -prep["cvec48"][:, 0] * prep["ainv48"][:, 0]
    return p


def _prep_generic(prep, mf_indices):
    mf = np.asarray(mf_indices)
    st = np.zeros((NQ, R), np.float32)
    for v in range(NIN):
        for m in range(NMF):
            st[v * 4 + m, mf[:, v] == m] = 1.0
    clhs = np.ascontiguousarray(
        prep["coeff_aug"].reshape(32, 128, 8).transpose(1, 0, 2)
        .reshape(128, 256))
    red8 = np.zeros((8, 2), np.float32)
    red8[:7, 0] = 1.0
    red8[7, 1] = 1.0
    return dict(st=st, clhs=clhs, red8=red8)


# ---------------------------------------------------------------------------
# device programs
# ---------------------------------------------------------------------------

def _build_fast16(b2):
    nc = _BaccOneActSet("TRN2", target_bir_lowering=False, debug=False,
                        enable_partition_id=False)
    rdt = F32R

    pcore_d = nc.dram_tensor("pcore", [48, 516], F32, kind="ExternalInput")
    xp8_d = nc.dram_tensor("xp8", [8, BL], F32, kind="ExternalInput")
    wf_d = nc.dram_tensor("wf", [128, 394], rdt, kind="ExternalInput")
    wb_d = nc.dram_tensor("wb", [48, 272], rdt, kind="ExternalInput")
    vout_d = nc.dram_tensor("vout", [8, BL], F32, kind="ExternalOutput")

    with tile.TileContext(nc) as tc:
        with (
            tc.tile_pool(name="sbuf", bufs=1) as pool,
            tc.tile_pool(name="psum", bufs=1, space="PSUM") as psum,
        ):
            pc = pool.tile([48, 516], F32)
            nc.sync.dma_start(pc[:], pcore_d[:, :])
            wb = pool.tile([48, 272], rdt)
            nc.scalar.dma_start(wb[:], wb_d[:, :])
            wf = pool.tile([128, 394], rdt)
            nc.gpsimd.dma_start(wf[:], wf_d[:, :])
            xp8 = pool.tile([8, BL], F32)
            nc.scalar.dma_start(xp8[:], xp8_d[:, :])

            # PE warm-up: ~3.5us of junk matmuls during the DMA/membership
            # stage flips the HAM clock gate to 8/8 (2.4 GHz) before the
            # real matmuls; cold f32r runs 2 cycles/row, warm runs 1.
            psW = psum.tile([16, 272], F32)
            for _ in range(7):
                nc.tensor.matmul(psW[:], wb[0:8, 0:16], wb[0:8, :])

            xT = pc[:, 0:512]
            ainv = pc[:, 513:514]
            bexp = pc[:, 514:515]
            negca = pc[:, 515:516]

            # dist = ((x-c)/a)^2 in one ACT op: Square(x*ainv - c*ainv)
            dist = pool.tile([48, BL], F32)
            nc.scalar.activation(dist[:], xT, AF.Square, bias=negca,
                                 scale=ainv)
            uu = pool.tile([48, BL], F32)
            if b2:
                # dist^b == dist^2 exactly when b == 2 everywhere
                nc.vector.tensor_tensor(uu[:], dist[:], dist[:], ALU.mult)
            else:
                lnd = pool.tile([48, BL], F32)
                nc.scalar.activation(lnd[:], dist[:], AF.Ln)
                nc.scalar.activation(uu[:], lnd[:], AF.Exp, scale=bexp)
            lpr = pool.tile([48, BL], rdt)
            nc.scalar.activation(lpr[:], uu[:], AF.Ln, bias=1.0)

            # A2 = exp(-(s3a2^T @ L01))
            psA2 = psum.tile([16, BL], F32)
            nc.tensor.matmul(psA2[:], wb[0:8, 0:16], lpr[0:8, :])
            a2sb = pool.tile([16, BL], rdt)
            nc.scalar.activation(a2sb[:], psA2[:], AF.Exp, scale=-1.0)

            # B4 = exp(-(s3b4^T @ L25)), two 128-row chunks
            b4sb = pool.tile([128, 2 * BL], rdt)
            for kk in range(2):
                psB4 = psum.tile([128, BL], F32, tag="psB4", bufs=2)
                nc.tensor.matmul(
                    psB4[:], wb[32:48, 16 + kk * 128:16 + (kk + 1) * 128],
                    lpr[32:48, :])
                nc.scalar.activation(b4sb[:, kk * BL:(kk + 1) * BL],
                                     psB4[:], AF.Exp, scale=-1.0)

            # ABig = ecat2^T @ A2 (broadcast A2 rows over j)
            psBig = psum.tile([128, BL], F32)
            nc.tensor.matmul(psBig[:], wf[0:16, 256:384], a2sb[:])
            abig = pool.tile([128, BL], rdt)
            nc.scalar.copy(abig[:], psBig[:])

            # T = sum_kk C4_kk^T @ B4_kk
            psT = psum.tile([128, BL], F32)
            for kk in range(2):
                nc.tensor.matmul(psT[:], wf[:, kk * 128:(kk + 1) * 128],
                                 b4sb[:, kk * BL:(kk + 1) * BL],
                                 start=(kk == 0), stop=(kk == 1))

            u8 = pool.tile([128, BL], rdt)
            nc.vector.tensor_tensor(u8[:], psT[:], abig[:], ALU.mult)
            psG = psum.tile([8, BL], F32)
            nc.tensor.matmul(psG[:], wf[:, 384:392], u8[:])
            v8 = pool.tile([8, BL], F32)
            nc.vector.tensor_tensor(v8[:], psG[:], xp8[:], ALU.mult)
            nc.sync.dma_start(vout_d[:, :], v8[:])

    nc.compile()
    return nc


def _build_generic():
    """One-hot log-space matmul over 32 rule tiles; handles any
    mf_indices and any exponent tensor b."""
    nc = _BaccOneActSet("TRN2", target_bir_lowering=False, debug=False,
                        enable_partition_id=False)
    rdt = F32R

    xT_d = nc.dram_tensor("xT24", [NQ, BL], F32, kind="ExternalInput")
    cvec_d = nc.dram_tensor("cvec", [NQ, 1], F32, kind="ExternalInput")
    ainv_d = nc.dram_tensor("ainv", [NQ, 1], F32, kind="ExternalInput")
    bexp_d = nc.dram_tensor("bexp", [NQ, 1], F32, kind="ExternalInput")
    xp8_d = nc.dram_tensor("xp8", [8, BL], F32, kind="ExternalInput")
    st_d = nc.dram_tensor("st", [NQ, R], BF16, kind="ExternalInput")
    clhs_d = nc.dram_tensor("clhs", [128, 256], rdt, kind="ExternalInput")
    red8_d = nc.dram_tensor("red8", [8, 2], rdt, kind="ExternalInput")
    out2_d = nc.dram_tensor("out2", [2, BL], F32, kind="ExternalOutput")

    with tile.TileContext(nc) as tc:
        with (
            tc.tile_pool(name="sbuf", bufs=1) as pool,
            tc.tile_pool(name="ps_r", bufs=4, space="PSUM") as ps_r,
            tc.tile_pool(name="ps_g", bufs=1, space="PSUM") as ps_g,
        ):
            xT = pool.tile([NQ, BL], F32)
            nc.sync.dma_start(xT[:], xT_d[:, :])
            cvec = pool.tile([NQ, 1], F32)
            nc.sync.dma_start(cvec[:], cvec_d[:, :])
            ainv = pool.tile([NQ, 1], F32)
            nc.sync.dma_start(ainv[:], ainv_d[:, :])
            bexp = pool.tile([NQ, 1], F32)
            nc.sync.dma_start(bexp[:], bexp_d[:, :])
            xp8 = pool.tile([8, BL], F32)
            nc.scalar.dma_start(xp8[:], xp8_d[:, :])
            stsb = pool.tile([NQ, R], BF16)
            nc.scalar.dma_start(stsb[:], st_d[:, :])
            clhs = pool.tile([128, 256], rdt)
            nc.sync.dma_start(clhs[:], clhs_d[:, :])
            red8 = pool.tile([8, 2], rdt)
            nc.scalar.dma_start(red8[:], red8_d[:, :])

            tdiff = pool.tile([NQ, BL], F32)
            nc.vector.tensor_scalar(tdiff[:], xT[:], cvec[:], ainv[:],
                                    ALU.subtract, ALU.mult)
            dist = pool.tile([NQ, BL], F32)
            nc.vector.tensor_tensor(dist[:], tdiff[:], tdiff[:], ALU.mult)
            lnd = pool.tile([NQ, BL], F32)
            nc.scalar.activation(lnd[:], dist[:], AF.Ln)
            uexp = pool.tile([NQ, BL], F32)
            nc.scalar.activation(uexp[:], lnd[:], AF.Exp, scale=bexp[:])
            lpos = pool.tile([NQ, BL], F32)
            nc.scalar.activation(lpos[:], uexp[:], AF.Ln, bias=1.0)
            lhi = pool.tile([NQ, BL], BF16)
            nc.vector.tensor_copy(lhi[:], lpos[:])
            llo = pool.tile([NQ, BL], BF16)
            nc.vector.tensor_tensor(llo[:], lpos[:], lhi[:], ALU.subtract)

            psumG = ps_g.tile([8, BL], F32)
            for k in range(32):
                psR = ps_r.tile([128, BL], F32, tag="psR")
                ssl = stsb[:, k * 128:(k + 1) * 128]
                nc.tensor.matmul(psR[:], ssl, lhi[:], start=True, stop=False)
                nc.tensor.matmul(psR[:], ssl, llo[:], start=False, stop=True)
                rt = pool.tile([128, BL], rdt, tag="rt", bufs=3)
                nc.scalar.activation(rt[:], psR[:], AF.Exp, scale=-1.0)
                nc.tensor.matmul(psumG[:], clhs[:, k * 8:(k + 1) * 8], rt[:],
                                 start=(k == 0), stop=(k == 31))

            v = pool.tile([8, BL], rdt)
            nc.vector.tensor_tensor(v[:], psumG[:], xp8[:], ALU.mult)
            psumO = ps_g.tile([2, BL], F32)
            nc.tensor.matmul(psumO[:], red8[:], v[:])
            out2 = pool.tile([2, BL], F32)
            nc.scalar.copy(out2[:], psumO[:])
            nc.sync.dma_start(out2_d[:, :], out2[:])

    nc.compile()
    return nc


# ---------------------------------------------------------------------------
# entry point
# ---------------------------------------------------------------------------

_cache = {}

# Test hook: when _TRACE is set (by an external harness that has NTFF
# profiling plumbed), run with tracing and stash the BassKernelResults.
_TRACE = False
_last_result = None


def _get_nc(key, builder):
    if key not in _cache:
        _cache[key] = builder()
    return _cache[key]


def kernel(x, a, b, c, coeff, mf_indices):
    x = np.asarray(x)
    prep = _prep_common(x, a, b, c, coeff)
    b2 = bool(np.all(np.asarray(b, np.float32) == 2.0))
    fast = _is_lex_product(mf_indices)

    out = np.empty((B, 1), np.float32)
    if fast:
        nc = _get_nc(("fast16", b2), lambda: _build_fast16(b2))
        f16 = _prep_fast16(prep)
        wf = np.ascontiguousarray(f16["wf"])
        wb = np.ascontiguousarray(f16["wb"])
        in_maps = []
        for core in range(NCORES):
            sl = slice(core * BL, (core + 1) * BL)
            in_maps.append({
                "pcore": _make_pcore(prep, sl),
                "xp8": np.ascontiguousarray(prep["xp8"][:, sl]),
                "wf": wf,
                "wb": wb,
            })
        res = run_bass_kernel_spmd(nc, in_maps, core_ids=list(range(NCORES)),
                                   trace=_TRACE)
        globals()["_last_result"] = res
        for core in range(NCORES):
            v = res.results[core]["vout"]
            out[core * BL:(core + 1) * BL, 0] = (
                v[:7].sum(axis=0) / np.maximum(v[7], 1e-12))
    else:
        nc = _get_nc(("generic",), _build_generic)
        g = _prep_generic(prep, mf_indices)
        st16 = np.ascontiguousarray(
            g["st"].astype(mybir.dt.np(BF16)))
        in_maps = []
        for core in range(NCORES):
            sl = slice(core * BL, (core + 1) * BL)
            in_maps.append({
                "xT24": np.ascontiguousarray(prep["xT24"][:, sl]),
                "cvec": prep["cvec"],
                "ainv": prep["ainv"],
                "bexp": prep["bexp"],
                "xp8": np.ascontiguousarray(prep["xp8"][:, sl]),
                "st": st16,
                "clhs": g["clhs"],
                "red8": g["red8"],
            })
        res = run_bass_kernel_spmd(nc, in_maps, core_ids=list(range(NCORES)),
                                   trace=_TRACE)
        globals()["_last_result"] = res
        for core in range(NCORES):
            o2 = res.results[core]["out2"]
            out[core * BL:(core + 1) * BL, 0] = (
                o2[0] / np.maximum(o2[1], 1e-12))
    return out


# revision 6
# speedup vs baseline: 1.0212x; 1.0212x over previous
"""TRN2 Bass kernel for nn_RF_ANFISModel (ANFIS forward pass).

Model (per batch row b):
  memb[b,v,m] = 1/(1+((x[b,v]-c[v,m])/a[v,m])^2 ^ b[v,m])
  rules[b,r]  = prod_v memb[b, v, mf_indices[r,v]]
  out[b]      = sum_r rules*(coeff[r,0,:].[x;1]) / max(sum_r rules, 1e-12)

Strategy: pure data parallelism over the batch (8 NeuronCores x 512 rows).
The rule product is computed in log space: rules = exp(-(S @ Lpos)) with
Lpos[(v,m),b] = ln(1+dist^b) and S a one-hot selection matrix.

Fast path (mf_indices == full lexicographic product, as produced by
setup_inputs): rules[r] factorizes as A2[hi2]*B4[lo4] with hi2=(i1,i2)
(16 values) and lo4=(i3..i6) (256 values). The coeff contraction
G[j,b] = sum_r coeff_aug[r,j]*rules[r,b] is restructured as
  T[(hi2,j),b]   = sum_lo4 C4[lo4,(hi2,j)] * B4[lo4,b]      (PE, K=256)
  ABig[(hi2,j),b]= A2[hi2,b]  (broadcast via one-hot matmul)
  U = T*ABig ; G[j,b] = sum_hi2 U[(hi2,j),b] ; V = G*xp8
and the host finishes with out = sum_j<7 V[j] / max(V[7], 1e-12).

Generic fallback (arbitrary mf_indices / non-2.0 exponents) does the
full one-hot log-space matmul over 32 rule tiles.

Value-domain matmuls run in float32r (~12-bit mantissa rounding of the
operands, fp32 accumulation): measured end-to-end max error ~3e-4 of the
output scale. Log-domain one-hot matmuls use a bf16 hi/lo split in the
generic path.
"""
import numpy as np

import concourse.mybir as mybir
import concourse.tile as tile
from concourse import bacc
import bass_rust as _bass_rust
from concourse.hw_specs import get_activation_tables
from concourse.bass_utils import run_bass_kernel_spmd

F32 = mybir.dt.float32
F32R = mybir.dt.float32r
BF16 = mybir.dt.bfloat16
AF = mybir.ActivationFunctionType
ALU = mybir.AluOpType

NIN, NMF, NQ, R = 6, 4, 24, 4096
NCORES, B, BL = 8, 4096, 512


class _BaccOneActSet(bacc.Bacc):
    """Bacc whose act-table pass only sees `natural_log_exp_and_others`
    (true set index preserved), so Ln/Exp/Copy/Square share one table
    load instead of thrashing between `natural_log` and
    `exp_and_others`."""

    _ONE_SET = "natural_log_exp_and_others"

    def insert_act_table_loads(self):
        has_activation = any(
            isinstance(i, mybir.InstActivation)
            for b in self.main_func.blocks
            for i in b.instructions
        )
        if not has_activation:
            return
        tables = [
            (name, funcs if name == self._ONE_SET else set())
            for name, funcs in get_activation_tables(self.m.arch).items()
        ]
        _bass_rust.insert_act_table_loads(self, tables)


# ---------------------------------------------------------------------------
# host-side prep
# ---------------------------------------------------------------------------

def _pad48(arr24):
    """[24, N] -> [48, N]: q-rows for v0,v1 at partitions 0:8; v2..v5 at
    32:48 (engine partition reads must be 32-aligned)."""
    out = np.tile(arr24[0:1], (48, 1)).astype(np.float32)
    out[0:8] = arr24[0:8]
    out[32:48] = arr24[8:24]
    return np.ascontiguousarray(out)


def _is_lex_product(mf_indices):
    r = np.arange(R)
    want = np.stack(
        [(r >> (2 * (NIN - 1 - v))) & 3 for v in range(NIN)], axis=1)
    return mf_indices.shape == (R, NIN) and np.array_equal(
        np.asarray(mf_indices).astype(np.int64), want)


def _prep_common(x, a, b, c, coeff):
    xT = np.ascontiguousarray(np.asarray(x, np.float32).T)     # [6, B]
    xT24 = np.repeat(xT, NMF, axis=0)                          # [24, B]
    cvec = np.asarray(c, np.float32).reshape(NQ, 1)
    ainv = (1.0 / np.asarray(a, np.float32)).reshape(NQ, 1)
    bexp = np.asarray(b, np.float32).reshape(NQ, 1)
    xp8 = np.ones((8, xT.shape[1]), np.float32)
    xp8[:NIN] = xT
    coeff_aug = np.ones((R, 8), np.float32)
    coeff_aug[:, :7] = np.asarray(coeff, np.float32).reshape(R, 7)
    return dict(xT24=xT24, cvec=cvec, ainv=ainv, bexp=bexp, xp8=xp8,
                coeff_aug=coeff_aug,
                x48=_pad48(xT24), cvec48=_pad48(cvec), ainv48=_pad48(ainv),
                bexp48=_pad48(bexp))


def _prep_fast16(prep):
    coeff_aug = prep["coeff_aug"]
    # wf [128, 394] f32r: cols 0:256 C4 (kk-major), 256:384 ecat2,
    # 384:392 red, 392:394 unused
    wf = np.zeros((128, 394), np.float32)
    c4 = coeff_aug.reshape(16, 2, 128, 8)   # [hi2, kk, lo4-within-chunk, j]
    for kk in range(2):
        for hi2 in range(16):
            for j in range(8):
                wf[:, kk * 128 + hi2 * 8 + j] = c4[hi2, kk, :, j]
    for hi2 in range(16):
        wf[hi2, 256 + hi2 * 8:256 + hi2 * 8 + 8] = 1.0
    wf[:, 384:392] = np.tile(np.eye(8, dtype=np.float32), (16, 1))

    # wb [48, 272]: rows 0:8 cols 0:16 s3a2 one-hot; rows 32:48 cols
    # 16:272 s3b4 one-hot
    wb = np.zeros((48, 272), np.float32)
    hi2 = np.arange(16)
    for v in range(2):
        dig = (hi2 >> (2 * (1 - v))) & 3
        for m in range(NMF):
            wb[v * 4 + m, np.where(dig == m)[0]] = 1.0
    lo4 = np.arange(256)
    for v in range(4):
        dig = (lo4 >> (2 * (3 - v))) & 3
        for m in range(NMF):
            wb[32 + v * 4 + m, 16 + np.where(dig == m)[0]] = 1.0
    return dict(wf=wf, wb=wb)


def _make_pcore(prep, sl):
    p = np.zeros((48, 516), np.float32)
    p[:, 0:512] = prep["x48"][:, sl]
    p[:, 512] = prep["cvec48"][:, 0]
    p[:, 513] = prep["ainv48"][:, 0]
    p[:, 514] = prep["bexp48"][:, 0]
    p[:, 515] = -prep["cvec48"][:, 0] * prep["ainv48"][:, 0]
    return p


def _prep_generic(prep, mf_indices):
    mf = np.asarray(mf_indices)
    st = np.zeros((NQ, R), np.float32)
    for v in range(NIN):
        for m in range(NMF):
            st[v * 4 + m, mf[:, v] == m] = 1.0
    clhs = np.ascontiguousarray(
        prep["coeff_aug"].reshape(32, 128, 8).transpose(1, 0, 2)
        .reshape(128, 256))
    red8 = np.zeros((8, 2), np.float32)
    red8[:7, 0] = 1.0
    red8[7, 1] = 1.0
    return dict(st=st, clhs=clhs, red8=red8)


# ---------------------------------------------------------------------------
# device programs
# ---------------------------------------------------------------------------

def _build_fast16(b2):
    nc = _BaccOneActSet("TRN2", target_bir_lowering=False, debug=False,
                        enable_partition_id=False)
    rdt = F32R

    pcore_d = nc.dram_tensor("pcore", [48, 516], F32, kind="ExternalInput")
    xp8_d = nc.dram_tensor("xp8", [8, BL], F32, kind="ExternalInput")
    wf_d = nc.dram_tensor("wf", [128, 394], rdt, kind="ExternalInput")
    wb_d = nc.dram_tensor("wb", [48, 272], rdt, kind="ExternalInput")
    vout_d = nc.dram_tensor("vout", [8, BL], F32, kind="ExternalOutput")

    with tile.TileContext(nc) as tc:
        with (
            tc.tile_pool(name="sbuf", bufs=1) as pool,
            tc.tile_pool(name="psum", bufs=1, space="PSUM") as psum,
        ):
            pc = pool.tile([48, 516], F32)
            nc.sync.dma_start(pc[:], pcore_d[:, :])
            wb = pool.tile([48, 272], rdt)
            nc.scalar.dma_start(wb[:], wb_d[:, :])
            wf = pool.tile([128, 394], rdt)
            nc.gpsimd.dma_start(wf[:], wf_d[:, :])
            xp8 = pool.tile([8, BL], F32)
            nc.scalar.dma_start(xp8[:], xp8_d[:, :])

            # PE warm-up: junk matmuls on a memset scratch tile keep the
            # PE busy from right after the preamble through the membership
            # stage, flipping the HAM clock gate to 8/8 (2.4 GHz) before
            # the real matmuls (cold f32r runs 2 cycles/row, warm runs 1).
            scr = pool.tile([16, 272], BF16)
            nc.vector.memset(scr[:], 1.0)
            psW = psum.tile([16, 272], F32)
            for _ in range(12):
                nc.tensor.matmul(psW[:], scr[0:8, 0:16], scr[0:8, :])

            xT = pc[:, 0:512]
            ainv = pc[:, 513:514]
            bexp = pc[:, 514:515]
            negca = pc[:, 515:516]

            # dist = ((x-c)/a)^2 in one ACT op: Square(x*ainv - c*ainv)
            dist = pool.tile([48, BL], F32)
            nc.scalar.activation(dist[:], xT, AF.Square, bias=negca,
                                 scale=ainv)
            uu = pool.tile([48, BL], F32)
            if b2:
                # dist^b == dist^2 exactly when b == 2 everywhere
                nc.vector.tensor_tensor(uu[:], dist[:], dist[:], ALU.mult)
            else:
                lnd = pool.tile([48, BL], F32)
                nc.scalar.activation(lnd[:], dist[:], AF.Ln)
                nc.scalar.activation(uu[:], lnd[:], AF.Exp, scale=bexp)
            lpr = pool.tile([48, BL], rdt)
            nc.scalar.activation(lpr[:], uu[:], AF.Ln, bias=1.0)

            # A2 = exp(-(s3a2^T @ L01))
            psA2 = psum.tile([16, BL], F32)
            nc.tensor.matmul(psA2[:], wb[0:8, 0:16], lpr[0:8, :])
            a2sb = pool.tile([16, BL], rdt)
            nc.scalar.activation(a2sb[:], psA2[:], AF.Exp, scale=-1.0)

            # B4 = exp(-(s3b4^T @ L25)), two 128-row chunks
            b4sb = pool.tile([128, 2 * BL], rdt)
            for kk in range(2):
                psB4 = psum.tile([128, BL], F32, tag="psB4", bufs=2)
                nc.tensor.matmul(
                    psB4[:], wb[32:48, 16 + kk * 128:16 + (kk + 1) * 128],
                    lpr[32:48, :])
                nc.scalar.activation(b4sb[:, kk * BL:(kk + 1) * BL],
                                     psB4[:], AF.Exp, scale=-1.0)

            # ABig = ecat2^T @ A2 (broadcast A2 rows over j)
            psBig = psum.tile([128, BL], F32)
            nc.tensor.matmul(psBig[:], wf[0:16, 256:384], a2sb[:])
            abig = pool.tile([128, BL], rdt)
            nc.vector.tensor_copy(abig[:], psBig[:])

            # T = sum_kk C4_kk^T @ B4_kk
            psT = psum.tile([128, BL], F32)
            for kk in range(2):
                nc.tensor.matmul(psT[:], wf[:, kk * 128:(kk + 1) * 128],
                                 b4sb[:, kk * BL:(kk + 1) * BL],
                                 start=(kk == 0), stop=(kk == 1))

            u8 = pool.tile([128, BL], rdt)
            nc.vector.tensor_tensor(u8[:], psT[:], abig[:], ALU.mult)
            psG = psum.tile([8, BL], F32)
            nc.tensor.matmul(psG[:], wf[:, 384:392], u8[:])
            v8 = pool.tile([8, BL], F32)
            nc.vector.tensor_tensor(v8[:], psG[:], xp8[:], ALU.mult)
            nc.sync.dma_start(vout_d[:, :], v8[:])

    nc.compile()
    return nc


def _build_generic():
    """One-hot log-space matmul over 32 rule tiles; handles any
    mf_indices and any exponent tensor b."""
    nc = _BaccOneActSet("TRN2", target_bir_lowering=False, debug=False,
                        enable_partition_id=False)
    rdt = F32R

    xT_d = nc.dram_tensor("xT24", [NQ, BL], F32, kind="ExternalInput")
    cvec_d = nc.dram_tensor("cvec", [NQ, 1], F32, kind="ExternalInput")
    ainv_d = nc.dram_tensor("ainv", [NQ, 1], F32, kind="ExternalInput")
    bexp_d = nc.dram_tensor("bexp", [NQ, 1], F32, kind="ExternalInput")
    xp8_d = nc.dram_tensor("xp8", [8, BL], F32, kind="ExternalInput")
    st_d = nc.dram_tensor("st", [NQ, R], BF16, kind="ExternalInput")
    clhs_d = nc.dram_tensor("clhs", [128, 256], rdt, kind="ExternalInput")
    red8_d = nc.dram_tensor("red8", [8, 2], rdt, kind="ExternalInput")
    out2_d = nc.dram_tensor("out2", [2, BL], F32, kind="ExternalOutput")

    with tile.TileContext(nc) as tc:
        with (
            tc.tile_pool(name="sbuf", bufs=1) as pool,
            tc.tile_pool(name="ps_r", bufs=4, space="PSUM") as ps_r,
            tc.tile_pool(name="ps_g", bufs=1, space="PSUM") as ps_g,
        ):
            xT = pool.tile([NQ, BL], F32)
            nc.sync.dma_start(xT[:], xT_d[:, :])
            cvec = pool.tile([NQ, 1], F32)
            nc.sync.dma_start(cvec[:], cvec_d[:, :])
            ainv = pool.tile([NQ, 1], F32)
            nc.sync.dma_start(ainv[:], ainv_d[:, :])
            bexp = pool.tile([NQ, 1], F32)
            nc.sync.dma_start(bexp[:], bexp_d[:, :])
            xp8 = pool.tile([8, BL], F32)
            nc.scalar.dma_start(xp8[:], xp8_d[:, :])
            stsb = pool.tile([NQ, R], BF16)
            nc.scalar.dma_start(stsb[:], st_d[:, :])
            clhs = pool.tile([128, 256], rdt)
            nc.sync.dma_start(clhs[:], clhs_d[:, :])
            red8 = pool.tile([8, 2], rdt)
            nc.scalar.dma_start(red8[:], red8_d[:, :])

            tdiff = pool.tile([NQ, BL], F32)
            nc.vector.tensor_scalar(tdiff[:], xT[:], cvec[:], ainv[:],
                                    ALU.subtract, ALU.mult)
            dist = pool.tile([NQ, BL], F32)
            nc.vector.tensor_tensor(dist[:], tdiff[:], tdiff[:], ALU.mult)
            lnd = pool.tile([NQ, BL], F32)
            nc.scalar.activation(lnd[:], dist[:], AF.Ln)
            uexp = pool.tile([NQ, BL], F32)
            nc.scalar.activation(uexp[:], lnd[:], AF.Exp, scale=bexp[:])
            lpos = pool.tile([NQ, BL], F32)
            nc.scalar.activation(lpos[:], uexp[:], AF.Ln, bias=1.0)
            lhi = pool.tile([NQ, BL], BF16)
            nc.vector.tensor_copy(lhi[:], lpos[:])
            llo = pool.tile([NQ, BL], BF16)
            nc.vector.tensor_tensor(llo[:], lpos[:], lhi[:], ALU.subtract)

            psumG = ps_g.tile([8, BL], F32)
            for k in range(32):
                psR = ps_r.tile([128, BL], F32, tag="psR")
                ssl = stsb[:, k * 128:(k + 1) * 128]
                nc.tensor.matmul(psR[:], ssl, lhi[:], start=True, stop=False)
                nc.tensor.matmul(psR[:], ssl, llo[:], start=False, stop=True)
                rt = pool.tile([128, BL], rdt, tag="rt", bufs=3)
                nc.scalar.activation(rt[:], psR[:], AF.Exp, scale=-1.0)
                nc.tensor.matmul(psumG[:], clhs[:, k * 8:(k + 1) * 8], rt[:],
                                 start=(k == 0), stop=(k == 31))

            v = pool.tile([8, BL], rdt)
            nc.vector.tensor_tensor(v[:], psumG[:], xp8[:], ALU.mult)
            psumO = ps_g.tile([2, BL], F32)
            nc.tensor.matmul(psumO[:], red8[:], v[:])
            out2 = pool.tile([2, BL], F32)
            nc.scalar.copy(out2[:], psumO[:])
            nc.sync.dma_start(out2_d[:, :], out2[:])

    nc.compile()
    return nc


# ---------------------------------------------------------------------------
# entry point
# ---------------------------------------------------------------------------

_cache = {}

# Test hook: when _TRACE is set (by an external harness that has NTFF
# profiling plumbed), run with tracing and stash the BassKernelResults.
_TRACE = False
_last_result = None


def _get_nc(key, builder):
    if key not in _cache:
        _cache[key] = builder()
    return _cache[key]


def kernel(x, a, b, c, coeff, mf_indices):
    x = np.asarray(x)
    prep = _prep_common(x, a, b, c, coeff)
    b2 = bool(np.all(np.asarray(b, np.float32) == 2.0))
    fast = _is_lex_product(mf_indices)

    out = np.empty((B, 1), np.float32)
    if fast:
        nc = _get_nc(("fast16", b2), lambda: _build_fast16(b2))
        f16 = _prep_fast16(prep)
        wf = np.ascontiguousarray(f16["wf"])
        wb = np.ascontiguousarray(f16["wb"])
        in_maps = []
        for core in range(NCORES):
            sl = slice(core * BL, (core + 1) * BL)
            in_maps.append({
                "pcore": _make_pcore(prep, sl),
                "xp8": np.ascontiguousarray(prep["xp8"][:, sl]),
                "wf": wf,
                "wb": wb,
            })
        res = run_bass_kernel_spmd(nc, in_maps, core_ids=list(range(NCORES)),
                                   trace=_TRACE)
        globals()["_last_result"] = res
        for core in range(NCORES):
            v = res.results[core]["vout"]
            out[core * BL:(core + 1) * BL, 0] = (
                v[:7].sum(axis=0) / np.maximum(v[7], 1e-12))
    else:
        nc = _get_nc(("generic",), _build_generic)
        g = _prep_generic(prep, mf_indices)
        st16 = np.ascontiguousarray(
            g["st"].astype(mybir.dt.np(BF16)))
        in_maps = []
        for core in range(NCORES):
            sl = slice(core * BL, (core + 1) * BL)
            in_maps.append({
                "xT24": np.ascontiguousarray(prep["xT24"][:, sl]),
                "cvec": prep["cvec"],
                "ainv": prep["ainv"],
                "bexp": prep["bexp"],
                "xp8": np.ascontiguousarray(prep["xp8"][:, sl]),
                "st": st16,
                "clhs": g["clhs"],
                "red8": g["red8"],
            })
        res = run_bass_kernel_spmd(nc, in_maps, core_ids=list(range(NCORES)),
                                   trace=_TRACE)
        globals()["_last_result"] = res
        for core in range(NCORES):
            o2 = res.results[core]["out2"]
            out[core * BL:(core + 1) * BL, 0] = (
                o2[0] / np.maximum(o2[1], 1e-12))
    return out


# revision 7
# speedup vs baseline: 1.0295x; 1.0082x over previous
"""TRN2 Bass kernel for nn_RF_ANFISModel (ANFIS forward pass).

Model (per batch row b):
  memb[b,v,m] = 1/(1+((x[b,v]-c[v,m])/a[v,m])^2 ^ b[v,m])
  rules[b,r]  = prod_v memb[b, v, mf_indices[r,v]]
  out[b]      = sum_r rules*(coeff[r,0,:].[x;1]) / max(sum_r rules, 1e-12)

Strategy: pure data parallelism over the batch (8 NeuronCores x 512 rows).
The rule product is computed in log space: rules = exp(-(S @ Lpos)) with
Lpos[(v,m),b] = ln(1+dist^b) and S a one-hot selection matrix.

Fast path (mf_indices == full lexicographic product, as produced by
setup_inputs): rules[r] factorizes as A2[hi2]*B4[lo4] with hi2=(i1,i2)
(16 values) and lo4=(i3..i6) (256 values). The coeff contraction
G[j,b] = sum_r coeff_aug[r,j]*rules[r,b] is restructured as
  T[(hi2,j),b]   = sum_lo4 C4[lo4,(hi2,j)] * B4[lo4,b]      (PE, K=256)
  ABig[(hi2,j),b]= A2[hi2,b]  (broadcast via one-hot matmul)
  U = T*ABig ; G[j,b] = sum_hi2 U[(hi2,j),b] ; V = G*xp8
and the host finishes with out = sum_j<7 V[j] / max(V[7], 1e-12).

Generic fallback (arbitrary mf_indices / non-2.0 exponents) does the
full one-hot log-space matmul over 32 rule tiles.

Value-domain matmuls run in float32r (~12-bit mantissa rounding of the
operands, fp32 accumulation): measured end-to-end max error ~3e-4 of the
output scale. Log-domain one-hot matmuls use a bf16 hi/lo split in the
generic path.
"""
import numpy as np

import concourse.mybir as mybir
import concourse.tile as tile
from concourse import bacc
import bass_rust as _bass_rust
from concourse.hw_specs import get_activation_tables
from concourse.bass_utils import run_bass_kernel_spmd

F32 = mybir.dt.float32
F32R = mybir.dt.float32r
BF16 = mybir.dt.bfloat16
AF = mybir.ActivationFunctionType
ALU = mybir.AluOpType

NIN, NMF, NQ, R = 6, 4, 24, 4096
NCORES, B, BL = 8, 4096, 512


class _BaccOneActSet(bacc.Bacc):
    """Bacc whose act-table pass only sees `natural_log_exp_and_others`
    (true set index preserved), so Ln/Exp/Copy/Square share one table
    load instead of thrashing between `natural_log` and
    `exp_and_others`."""

    _ONE_SET = "natural_log_exp_and_others"

    def insert_act_table_loads(self):
        has_activation = any(
            isinstance(i, mybir.InstActivation)
            for b in self.main_func.blocks
            for i in b.instructions
        )
        if not has_activation:
            return
        tables = [
            (name, funcs if name == self._ONE_SET else set())
            for name, funcs in get_activation_tables(self.m.arch).items()
        ]
        _bass_rust.insert_act_table_loads(self, tables)


# ---------------------------------------------------------------------------
# host-side prep
# ---------------------------------------------------------------------------

def _pad48(arr24):
    """[24, N] -> [48, N]: q-rows for v0,v1 at partitions 0:8; v2..v5 at
    32:48 (engine partition reads must be 32-aligned)."""
    out = np.tile(arr24[0:1], (48, 1)).astype(np.float32)
    out[0:8] = arr24[0:8]
    out[32:48] = arr24[8:24]
    return np.ascontiguousarray(out)


def _is_lex_product(mf_indices):
    r = np.arange(R)
    want = np.stack(
        [(r >> (2 * (NIN - 1 - v))) & 3 for v in range(NIN)], axis=1)
    return mf_indices.shape == (R, NIN) and np.array_equal(
        np.asarray(mf_indices).astype(np.int64), want)


def _prep_common(x, a, b, c, coeff):
    xT = np.ascontiguousarray(np.asarray(x, np.float32).T)     # [6, B]
    xT24 = np.repeat(xT, NMF, axis=0)                          # [24, B]
    cvec = np.asarray(c, np.float32).reshape(NQ, 1)
    ainv = (1.0 / np.asarray(a, np.float32)).reshape(NQ, 1)
    bexp = np.asarray(b, np.float32).reshape(NQ, 1)
    xp8 = np.ones((8, xT.shape[1]), np.float32)
    xp8[:NIN] = xT
    coeff_aug = np.ones((R, 8), np.float32)
    coeff_aug[:, :7] = np.asarray(coeff, np.float32).reshape(R, 7)
    return dict(xT24=xT24, cvec=cvec, ainv=ainv, bexp=bexp, xp8=xp8,
                coeff_aug=coeff_aug,
                x48=_pad48(xT24), cvec48=_pad48(cvec), ainv48=_pad48(ainv),
                bexp48=_pad48(bexp))


def _prep_fast16(prep):
    coeff_aug = prep["coeff_aug"]
    # wf [128, 394] f32r: cols 0:256 C4 (kk-major), 256:384 ecat2,
    # 384:392 red, 392:394 unused
    wf = np.zeros((128, 394), np.float32)
    c4 = coeff_aug.reshape(16, 2, 128, 8)   # [hi2, kk, lo4-within-chunk, j]
    for kk in range(2):
        for hi2 in range(16):
            for j in range(8):
                wf[:, kk * 128 + hi2 * 8 + j] = c4[hi2, kk, :, j]
    for hi2 in range(16):
        wf[hi2, 256 + hi2 * 8:256 + hi2 * 8 + 8] = 1.0
    wf[:, 384:392] = np.tile(np.eye(8, dtype=np.float32), (16, 1))

    # wb [48, 272]: rows 0:8 cols 0:16 s3a2 one-hot; rows 32:48 cols
    # 16:272 s3b4 one-hot
    wb = np.zeros((48, 272), np.float32)
    hi2 = np.arange(16)
    for v in range(2):
        dig = (hi2 >> (2 * (1 - v))) & 3
        for m in range(NMF):
            wb[v * 4 + m, np.where(dig == m)[0]] = 1.0
    lo4 = np.arange(256)
    for v in range(4):
        dig = (lo4 >> (2 * (3 - v))) & 3
        for m in range(NMF):
            wb[32 + v * 4 + m, 16 + np.where(dig == m)[0]] = 1.0
    return dict(wf=wf, wb=wb)


def _make_pcore(prep, sl):
    p = np.zeros((48, 516), np.float32)
    p[:, 0:512] = prep["x48"][:, sl]
    p[:, 512] = prep["cvec48"][:, 0]
    p[:, 513] = prep["ainv48"][:, 0]
    p[:, 514] = prep["bexp48"][:, 0]
    p[:, 515] = -prep["cvec48"][:, 0] * prep["ainv48"][:, 0]
    return p


def _prep_generic(prep, mf_indices):
    mf = np.asarray(mf_indices)
    st = np.zeros((NQ, R), np.float32)
    for v in range(NIN):
        for m in range(NMF):
            st[v * 4 + m, mf[:, v] == m] = 1.0
    clhs = np.ascontiguousarray(
        prep["coeff_aug"].reshape(32, 128, 8).transpose(1, 0, 2)
        .reshape(128, 256))
    red8 = np.zeros((8, 2), np.float32)
    red8[:7, 0] = 1.0
    red8[7, 1] = 1.0
    return dict(st=st, clhs=clhs, red8=red8)


# ---------------------------------------------------------------------------
# device programs
# ---------------------------------------------------------------------------

def _build_fast16(b2):
    nc = _BaccOneActSet("TRN2", target_bir_lowering=False, debug=False,
                        enable_partition_id=False)
    rdt = F32R

    pcore_d = nc.dram_tensor("pcore", [48, 516], F32, kind="ExternalInput")
    xp8_d = nc.dram_tensor("xp8", [8, BL], F32, kind="ExternalInput")
    wf_d = nc.dram_tensor("wf", [128, 394], rdt, kind="ExternalInput")
    wb_d = nc.dram_tensor("wb", [48, 272], rdt, kind="ExternalInput")
    vout_d = nc.dram_tensor("vout", [8, BL], F32, kind="ExternalOutput")

    with tile.TileContext(nc) as tc:
        with (
            tc.tile_pool(name="sbuf", bufs=1) as pool,
            tc.tile_pool(name="psum", bufs=1, space="PSUM") as psum,
        ):
            pc = pool.tile([48, 516], F32)
            nc.sync.dma_start(pc[:], pcore_d[:, :])
            wb = pool.tile([48, 272], rdt)
            nc.scalar.dma_start(wb[:], wb_d[:, :])
            wf = pool.tile([128, 394], rdt)
            nc.gpsimd.dma_start(wf[:], wf_d[:, :])
            xp8 = pool.tile([8, BL], F32)
            nc.scalar.dma_start(xp8[:], xp8_d[:, :])

            # PE warm-up: junk matmuls on a memset scratch tile keep the
            # PE busy from right after the preamble through the membership
            # stage, flipping the HAM clock gate to 8/8 (2.4 GHz) before
            # the real matmuls (cold f32r runs 2 cycles/row, warm runs 1).
            scr = pool.tile([16, 272], BF16)
            nc.vector.memset(scr[:], 1.0)
            psW = psum.tile([16, 272], F32)
            for _ in range(16):
                nc.tensor.matmul(psW[:], scr[0:8, 0:16], scr[0:8, :])

            xT = pc[:, 0:512]
            ainv = pc[:, 513:514]
            bexp = pc[:, 514:515]
            negca = pc[:, 515:516]

            # dist = ((x-c)/a)^2 in one ACT op: Square(x*ainv - c*ainv)
            dist = pool.tile([48, BL], F32)
            nc.scalar.activation(dist[:], xT, AF.Square, bias=negca,
                                 scale=ainv)
            uu = pool.tile([48, BL], F32)
            if b2:
                # dist^b == dist^2 exactly when b == 2 everywhere
                nc.vector.tensor_tensor(uu[:], dist[:], dist[:], ALU.mult)
            else:
                lnd = pool.tile([48, BL], F32)
                nc.scalar.activation(lnd[:], dist[:], AF.Ln)
                nc.scalar.activation(uu[:], lnd[:], AF.Exp, scale=bexp)
            lpr = pool.tile([48, BL], rdt)
            nc.scalar.activation(lpr[:], uu[:], AF.Ln, bias=1.0)

            # A2 = exp(-(s3a2^T @ L01))
            psA2 = psum.tile([16, BL], F32)
            nc.tensor.matmul(psA2[:], wb[0:8, 0:16], lpr[0:8, :])
            a2sb = pool.tile([16, BL], rdt)
            nc.scalar.activation(a2sb[:], psA2[:], AF.Exp, scale=-1.0)

            # B4 = exp(-(s3b4^T @ L25)), two 128-row chunks
            b4sb = pool.tile([128, 2 * BL], rdt)
            for kk in range(2):
                psB4 = psum.tile([128, BL], F32, tag="psB4", bufs=2)
                nc.tensor.matmul(
                    psB4[:], wb[32:48, 16 + kk * 128:16 + (kk + 1) * 128],
                    lpr[32:48, :])
                nc.scalar.activation(b4sb[:, kk * BL:(kk + 1) * BL],
                                     psB4[:], AF.Exp, scale=-1.0)

            # ABig = ecat2^T @ A2 (broadcast A2 rows over j)
            psBig = psum.tile([128, BL], F32)
            nc.tensor.matmul(psBig[:], wf[0:16, 256:384], a2sb[:])
            abig = pool.tile([128, BL], rdt)
            nc.vector.tensor_copy(abig[:], psBig[:])

            # T = sum_kk C4_kk^T @ B4_kk
            psT = psum.tile([128, BL], F32)
            for kk in range(2):
                nc.tensor.matmul(psT[:], wf[:, kk * 128:(kk + 1) * 128],
                                 b4sb[:, kk * BL:(kk + 1) * BL],
                                 start=(kk == 0), stop=(kk == 1))

            u8 = pool.tile([128, BL], rdt)
            nc.vector.tensor_tensor(u8[:], psT[:], abig[:], ALU.mult)
            psG = psum.tile([8, BL], F32)
            nc.tensor.matmul(psG[:], wf[:, 384:392], u8[:])
            v8 = pool.tile([8, BL], F32)
            nc.vector.tensor_tensor(v8[:], psG[:], xp8[:], ALU.mult)
            nc.sync.dma_start(vout_d[:, :], v8[:])

    nc.compile()
    return nc


def _build_generic():
    """One-hot log-space matmul over 32 rule tiles; handles any
    mf_indices and any exponent tensor b."""
    nc = _BaccOneActSet("TRN2", target_bir_lowering=False, debug=False,
                        enable_partition_id=False)
    rdt = F32R

    xT_d = nc.dram_tensor("xT24", [NQ, BL], F32, kind="ExternalInput")
    cvec_d = nc.dram_tensor("cvec", [NQ, 1], F32, kind="ExternalInput")
    ainv_d = nc.dram_tensor("ainv", [NQ, 1], F32, kind="ExternalInput")
    bexp_d = nc.dram_tensor("bexp", [NQ, 1], F32, kind="ExternalInput")
    xp8_d = nc.dram_tensor("xp8", [8, BL], F32, kind="ExternalInput")
    st_d = nc.dram_tensor("st", [NQ, R], BF16, kind="ExternalInput")
    clhs_d = nc.dram_tensor("clhs", [128, 256], rdt, kind="ExternalInput")
    red8_d = nc.dram_tensor("red8", [8, 2], rdt, kind="ExternalInput")
    out2_d = nc.dram_tensor("out2", [2, BL], F32, kind="ExternalOutput")

    with tile.TileContext(nc) as tc:
        with (
            tc.tile_pool(name="sbuf", bufs=1) as pool,
            tc.tile_pool(name="ps_r", bufs=4, space="PSUM") as ps_r,
            tc.tile_pool(name="ps_g", bufs=1, space="PSUM") as ps_g,
        ):
            xT = pool.tile([NQ, BL], F32)
            nc.sync.dma_start(xT[:], xT_d[:, :])
            cvec = pool.tile([NQ, 1], F32)
            nc.sync.dma_start(cvec[:], cvec_d[:, :])
            ainv = pool.tile([NQ, 1], F32)
            nc.sync.dma_start(ainv[:], ainv_d[:, :])
            bexp = pool.tile([NQ, 1], F32)
            nc.sync.dma_start(bexp[:], bexp_d[:, :])
            xp8 = pool.tile([8, BL], F32)
            nc.scalar.dma_start(xp8[:], xp8_d[:, :])
            stsb = pool.tile([NQ, R], BF16)
            nc.scalar.dma_start(stsb[:], st_d[:, :])
            clhs = pool.tile([128, 256], rdt)
            nc.sync.dma_start(clhs[:], clhs_d[:, :])
            red8 = pool.tile([8, 2], rdt)
            nc.scalar.dma_start(red8[:], red8_d[:, :])

            tdiff = pool.tile([NQ, BL], F32)
            nc.vector.tensor_scalar(tdiff[:], xT[:], cvec[:], ainv[:],
                                    ALU.subtract, ALU.mult)
            dist = pool.tile([NQ, BL], F32)
            nc.vector.tensor_tensor(dist[:], tdiff[:], tdiff[:], ALU.mult)
            lnd = pool.tile([NQ, BL], F32)
            nc.scalar.activation(lnd[:], dist[:], AF.Ln)
            uexp = pool.tile([NQ, BL], F32)
            nc.scalar.activation(uexp[:], lnd[:], AF.Exp, scale=bexp[:])
            lpos = pool.tile([NQ, BL], F32)
            nc.scalar.activation(lpos[:], uexp[:], AF.Ln, bias=1.0)
            lhi = pool.tile([NQ, BL], BF16)
            nc.vector.tensor_copy(lhi[:], lpos[:])
            llo = pool.tile([NQ, BL], BF16)
            nc.vector.tensor_tensor(llo[:], lpos[:], lhi[:], ALU.subtract)

            psumG = ps_g.tile([8, BL], F32)
            for k in range(32):
                psR = ps_r.tile([128, BL], F32, tag="psR")
                ssl = stsb[:, k * 128:(k + 1) * 128]
                nc.tensor.matmul(psR[:], ssl, lhi[:], start=True, stop=False)
                nc.tensor.matmul(psR[:], ssl, llo[:], start=False, stop=True)
                rt = pool.tile([128, BL], rdt, tag="rt", bufs=3)
                nc.scalar.activation(rt[:], psR[:], AF.Exp, scale=-1.0)
                nc.tensor.matmul(psumG[:], clhs[:, k * 8:(k + 1) * 8], rt[:],
                                 start=(k == 0), stop=(k == 31))

            v = pool.tile([8, BL], rdt)
            nc.vector.tensor_tensor(v[:], psumG[:], xp8[:], ALU.mult)
            psumO = ps_g.tile([2, BL], F32)
            nc.tensor.matmul(psumO[:], red8[:], v[:])
            out2 = pool.tile([2, BL], F32)
            nc.scalar.copy(out2[:], psumO[:])
            nc.sync.dma_start(out2_d[:, :], out2[:])

    nc.compile()
    return nc


# ---------------------------------------------------------------------------
# entry point
# ---------------------------------------------------------------------------

_cache = {}

# Test hook: when _TRACE is set (by an external harness that has NTFF
# profiling plumbed), run with tracing and stash the BassKernelResults.
_TRACE = False
_last_result = None


def _get_nc(key, builder):
    if key not in _cache:
        _cache[key] = builder()
    return _cache[key]


def kernel(x, a, b, c, coeff, mf_indices):
    x = np.asarray(x)
    prep = _prep_common(x, a, b, c, coeff)
    b2 = bool(np.all(np.asarray(b, np.float32) == 2.0))
    fast = _is_lex_product(mf_indices)

    out = np.empty((B, 1), np.float32)
    if fast:
        nc = _get_nc(("fast16", b2), lambda: _build_fast16(b2))
        f16 = _prep_fast16(prep)
        wf = np.ascontiguousarray(f16["wf"])
        wb = np.ascontiguousarray(f16["wb"])
        in_maps = []
        for core in range(NCORES):
            sl = slice(core * BL, (core + 1) * BL)
            in_maps.append({
                "pcore": _make_pcore(prep, sl),
                "xp8": np.ascontiguousarray(prep["xp8"][:, sl]),
                "wf": wf,
                "wb": wb,
            })
        res = run_bass_kernel_spmd(nc, in_maps, core_ids=list(range(NCORES)),
                                   trace=_TRACE)
        globals()["_last_result"] = res
        for core in range(NCORES):
            v = res.results[core]["vout"]
            out[core * BL:(core + 1) * BL, 0] = (
                v[:7].sum(axis=0) / np.maximum(v[7], 1e-12))
    else:
        nc = _get_nc(("generic",), _build_generic)
        g = _prep_generic(prep, mf_indices)
        st16 = np.ascontiguousarray(
            g["st"].astype(mybir.dt.np(BF16)))
        in_maps = []
        for core in range(NCORES):
            sl = slice(core * BL, (core + 1) * BL)
            in_maps.append({
                "xT24": np.ascontiguousarray(prep["xT24"][:, sl]),
                "cvec": prep["cvec"],
                "ainv": prep["ainv"],
                "bexp": prep["bexp"],
                "xp8": np.ascontiguousarray(prep["xp8"][:, sl]),
                "st": st16,
                "clhs": g["clhs"],
                "red8": g["red8"],
            })
        res = run_bass_kernel_spmd(nc, in_maps, core_ids=list(range(NCORES)),
                                   trace=_TRACE)
        globals()["_last_result"] = res
        for core in range(NCORES):
            o2 = res.results[core]["out2"]
            out[core * BL:(core + 1) * BL, 0] = (
                o2[0] / np.maximum(o2[1], 1e-12))
    return out
